# revision 1
# baseline (speedup 1.0000x reference)
"""Causal self-attention (B=2, S=2048, D=1024, H=16) on 8 NeuronCores.

Sharding (per spec hint): data-parallel over batch (2 groups of 4 cores),
tensor-parallel over heads within a group (4 heads / core). Each core
computes Q/K/V projections for its 4 heads, causal flash-style attention,
and a partial output projection through its slice of Wo. The 4 partial
[2048, 1024] outputs per batch are summed on the host (unsharding step).

Per-core kernel layout notes:
  - All activations kept feature-major ("transposed"): xT [1024, 2048],
    QT/KT [256, 2048]. Scores are computed transposed, ST[k, q], so the
    P@V contraction (over k) needs no transposes anywhere.
  - Softmax skips the max-subtraction (scores ~ N(0,1); exp can't
    overflow fp32) so exp is a single ACT pass straight out of PSUM.
  - The softmax denominator rides along the P@V matmul as a fused
    ones-column in the V operand (M=65), then gets broadcast across
    partitions with a K=1 matmul for the normalize multiply.
  - Matmuls run in float32r (TF32-like, full PE rate; measured rms rel
    err 1.5e-4 at K=1024 on HW) with fp32 PSUM accumulation.
"""

import numpy as np

import concourse.bass as bass
import concourse.mybir as mybir
import concourse.tile as tile
from concourse.bass_utils import run_bass_kernel_spmd

F32 = mybir.dt.float32
F32R = mybir.dt.float32r
AF = mybir.ActivationFunctionType

B, S, D, H = 2, 2048, 1024, 16
DH = D // H              # 64
HL = 4                   # heads per core
CL = HL * DH             # 256 channels per core
G = 4                    # cores per batch group
SCALE = DH ** -0.5       # 0.125
NQC = S // 512           # 4 q-chunks of 512
NKT = S // 128           # 16 key tiles of 128


def _split_excess_waits(nc, max_waits=1):
    """walrus in this toolchain rejects instructions carrying more than
    `max_waits` sem waits; split the excess onto preceding same-engine
    NoOps (sound: waits are monotone >= conditions hoisted earlier on
    the same engine)."""
    n_split = 0
    for f in nc.m.functions:
        for bb in f.blocks:
            out = []
            for inst in bb.instructions:
                si = inst.sync_info
                waits = list(si.on_wait) if si is not None and si.on_wait else []
                if len(waits) > max_waits:
                    head, keep = waits[:-max_waits], waits[-max_waits:]
                    for ci, start in enumerate(range(0, len(head), max_waits)):
                        nop = mybir.InstNoOp(
                            name=f"{inst.name}_wsplit{ci}",
                            sync_info=mybir.SyncInfo(
                                on_wait=head[start:start + max_waits],
                                on_update=[],
                            ),
                            engine=inst.engine,
                            bass_nofuse=True,
                        )
                        out.append(nop)
                        n_split += 1
                    si.on_wait = keep
                out.append(inst)
            if n_split:
                bb.instructions.clear()
                for i in out:
                    bb.instructions.append(i)
    return n_split


def _build_nc(split_waits=True):
    nc = bass.Bass()
    xt_d = nc.dram_tensor("xt", [D, S], F32, kind="ExternalInput")
    wq_d = nc.dram_tensor("wq", [D, CL], F32, kind="ExternalInput")
    wk_d = nc.dram_tensor("wk", [D, CL], F32, kind="ExternalInput")
    wv_d = nc.dram_tensor("wv", [D, CL], F32, kind="ExternalInput")
    wo_d = nc.dram_tensor("wo", [128, 2, D], F32, kind="ExternalInput")
    mask_d = nc.dram_tensor("mask", [128, 2, 128], F32, kind="ExternalInput")
    y_d = nc.dram_tensor("y", [S, D], F32, kind="ExternalOutput")

    with tile.TileContext(nc) as tc:
        with tc.tile_pool(name="persist", bufs=1) as pp:
            # ---- persistent SBUF tensors -------------------------------
            wo_sb = pp.tile([128, 2, D], F32R)   # pair-major k-tiles
            mask_sb = pp.tile([128, 2, 128], F32)     # tri m[k,q]=k<=q, x2 heads
            ones_f = pp.tile([128, 128], F32)
            zeros_f = pp.tile([128, 768], F32)
            ones_sb = pp.tile([128, 128], F32R)
            qt_sb = [pp.tile([128, S], F32R, name=f"qt{p}", tag=f"qt{p}")
                     for p in range(2)]
            kt_sb = [pp.tile([128, S], F32R, name=f"kt{p}", tag=f"kt{p}")
                     for p in range(2)]
            # V' per key-tile: 4x[64 v-cols + 1 ones-col]
            vp_sb = pp.tile([128, NKT, 4 * 65], F32R)

            nc.vector.memset(ones_f[:], 1.0)
            nc.vector.memset(zeros_f[:], 0.0)
            nc.vector.tensor_copy(ones_sb[:], ones_f[:])
            for hl in range(4):
                nc.vector.tensor_copy(
                    vp_sb[:, :, hl * 65 + 64:hl * 65 + 65], ones_f[:, 0:NKT])

            # ---- phase 1: projections ---------------------------------
            # QT/KT [256, S] = W.T-slice.T @ xT ; V [S, 256] = xT.T @ wv
            with (
                tc.tile_pool(name="ph1", bufs=1) as ph1,
                tc.tile_pool(name="pj", bufs=2, space="PSUM") as pj,
            ):
                xt_sb = ph1.tile([128, 8, S], F32R)        # x.T, k-tiled
                wq_sb = ph1.tile([128, 8, CL], F32R)
                wk_sb = ph1.tile([128, 8, CL], F32R)
                wv_sb = ph1.tile([128, 8, CL], F32R)
                xt_r = xt_d.rearrange("(a p) s -> p a s", p=128).bitcast(F32R)
                nc.sync.dma_start(
                    wq_sb[:],
                    wq_d.rearrange("(a p) m -> p a m", p=128).bitcast(F32R))
                nc.sync.dma_start(
                    wk_sb[:],
                    wk_d.rearrange("(a p) m -> p a m", p=128).bitcast(F32R))
                nc.sync.dma_start(
                    wv_sb[:],
                    wv_d.rearrange("(a p) m -> p a m", p=128).bitcast(F32R))
                for k in range(8):  # per-k chunks so matmuls start early
                    nc.sync.dma_start(xt_sb[:, k, :], xt_r[:, k, :])
                # wo/mask are not needed until mid-attention; load after xT
                nc.sync.dma_start(wo_sb[:], wo_d[:, :, :].bitcast(F32R))
                nc.sync.dma_start(mask_sb[:], mask_d[:, :, :])
                # q-chunk-major so attention on early chunks starts sooner
                for c in range(NQC):
                    cslc = slice(c * 512, (c + 1) * 512)
                    for p in range(2):
                        pslc = slice(p * 128, (p + 1) * 128)
                        psq = pj.tile([128, 512], F32, tag="pq", bufs=3)
                        psk = pj.tile([128, 512], F32, tag="pk", bufs=3)
                        for k in range(8):
                            nc.tensor.matmul(
                                psq[:], wq_sb[:, k, pslc], xt_sb[:, k, cslc],
                                start=(k == 0), stop=(k == 7))
                        for k in range(8):
                            nc.tensor.matmul(
                                psk[:], wk_sb[:, k, pslc], xt_sb[:, k, cslc],
                                start=(k == 0), stop=(k == 7))
                        nc.scalar.copy(qt_sb[p][:, cslc], psq[:])
                        nc.scalar.copy(kt_sb[p][:, cslc], psk[:])
                    for st in range(4 * c, 4 * (c + 1)):
                        psv = pj.tile([128, CL], F32, tag="pv")
                        for k in range(8):
                            nc.tensor.matmul(
                                psv[:], xt_sb[:, k, st * 128:(st + 1) * 128],
                                wv_sb[:, k, :], start=(k == 0), stop=(k == 7))
                        nc.scalar.copy(
                            vp_sb[:, st, :]
                            .rearrange("p (h e) -> p h e", e=65)[:, :, 0:64],
                            psv[:].rearrange("p (h d) -> p h d", d=64))

            # ---- phase 2/3: attention + out-projection ----------------
            with (
                tc.tile_pool(name="stp", bufs=2, space="PSUM") as stp,
                tc.tile_pool(name="otp", bufs=2, space="PSUM") as otp,
                tc.tile_pool(name="pt", bufs=6) as ptp,
                tc.tile_pool(name="nrm", bufs=2) as nrm,
                tc.tile_pool(name="osb", bufs=4) as osb,
            ):
                for qc in range(NQC):
                    qlo = qc * 512
                    qslc = slice(qlo, qlo + 512)
                    os_tiles = []           # one [64, 512] tile per local head
                    for p in range(2):
                        OTP = otp.tile([65, 2, 512], F32, tag="ot")
                        ktmax = 4 * (qc + 1)
                        for kt in range(ktmax):
                            first, last = kt == 0, kt == ktmax - 1
                            ST = stp.tile([128, 2, 512], F32, tag="st")
                            for hi in range(2):
                                hslc = slice(hi * 64, (hi + 1) * 64)
                                nc.tensor.matmul(
                                    ST[:, hi, :],
                                    kt_sb[p][hslc, kt * 128:(kt + 1) * 128],
                                    qt_sb[p][hslc, qslc],
                                    start=True, stop=True)
                            PT = ptp.tile([128, 2, 512], F32R, tag="pt")
                            dq = max(0, kt * 128 - qlo)
                            nc.scalar.activation(PT[:, :, dq:], ST[:, :, dq:],
                                                 AF.Exp, scale=SCALE)
                            if kt * 128 >= qlo:  # diagonal: mask keys > query
                                if dq > 0:
                                    nc.vector.tensor_copy(
                                        PT[:, :, 0:dq], zeros_f[:, 0:2 * dq])
                                nc.vector.tensor_mul(
                                    PT[:, :, dq:dq + 128],
                                    PT[:, :, dq:dq + 128], mask_sb[:])
                            # P@V (transposed): OT[c, q] += [V|1].T @ PT
                            # row 64 of each head region = softmax denominator
                            for hi in range(2):
                                bc = (2 * p + hi) * 65
                                nc.tensor.matmul(
                                    OTP[0:65, hi, :], vp_sb[:, kt, bc:bc + 65],
                                    PT[:, hi, :], start=first, stop=last)
                        # normalize: rows 0:64 of each head / its denom row 64
                        Ri = nrm.tile([128, 2, 512], F32R, tag="ri")
                        with nc.allow_low_precision(reason="softmax recip"):
                            nc.vector.reciprocal(Ri[64:65, :, :],
                                                 OTP[64:65, :, :])
                        OC = osb.tile([64, 2, 512], F32, tag="oc")
                        nc.vector.tensor_copy(OC[:, :, :], OTP[0:64, :, :])
                        # pack the head pair into one [128, 512] k-tile for
                        # the out-projection: even head normalizes in place,
                        # odd head normalizes to a scratch tile and is moved
                        # to partitions 64:128 by an SBUF-to-SBUF DMA.
                        OS = osb.tile([128, 512], F32R, name="OS", tag=f"os{p}")
                        OSm = osb.tile([64, 512], F32R, name="OSm", tag="osm")
                        Rb = stp.tile([128, 2, 512], F32, name="Rb", tag="st")
                        for hi in range(2):
                            nc.tensor.matmul(
                                Rb[:, hi, :], ones_sb[64:65, :],
                                Ri[64:65, hi, :], start=True, stop=True)
                        nc.vector.tensor_mul(OS[0:64, :], OC[:, 0, :],
                                             Rb[0:64, 0, :])
                        nc.vector.tensor_mul(OSm[:, :], OC[:, 1, :],
                                             Rb[0:64, 1, :])
                        nc.sync.dma_start(OS[64:128, :], OSm[:, :])
                        os_tiles.append(OS)
                    # out-projection for this q-chunk: accumulate over 2 pairs
                    for st4 in range(4):
                        sslc = slice(st4 * 128, (st4 + 1) * 128)
                        for nch in range(2):
                            yp = otp.tile([128, 512], F32, name="yp", tag="ot")
                            for kp in range(2):
                                nc.tensor.matmul(
                                    yp[:], os_tiles[kp][:, sslc],
                                    wo_sb[:, kp, nch * 512:(nch + 1) * 512],
                                    start=(kp == 0), stop=(kp == 1))
                            ysb = osb.tile([128, 512], F32, name="ysb", tag="ys")
                            nc.vector.tensor_copy(ysb[:], yp[:])
                            nc.sync.dma_start(
                                y_d[qlo + st4 * 128:qlo + (st4 + 1) * 128,
                                    nch * 512:(nch + 1) * 512], ysb[:])

    if split_waits:
        _split_excess_waits(nc, max_waits=1)
    return nc


_NC = None


def kernel(x, Wq, Wk, Wv, Wo):
    global _NC
    if _NC is None:
        _NC = _build_nc()
    x = np.asarray(x, dtype=np.float32)
    Wq, Wk, Wv, Wo = (np.asarray(w, dtype=np.float32) for w in (Wq, Wk, Wv, Wo))

    tri = np.triu(np.ones((128, 128), dtype=np.float32))  # m[k,q] = k<=q
    in_maps = []
    for core in range(8):
        b, g = divmod(core, G)
        csl = slice(g * CL, (g + 1) * CL)
        in_maps.append({
            "xt": np.ascontiguousarray(x[b].T),
            "wq": np.ascontiguousarray(Wq[csl, :].T),
            "wk": np.ascontiguousarray(Wk[csl, :].T),
            "wv": np.ascontiguousarray(Wv[csl, :].T),
            "wo": np.ascontiguousarray(
                Wo[:, csl].T.reshape(2, 128, D).transpose(1, 0, 2)),
            "mask": np.ascontiguousarray(np.stack([tri, tri], axis=1)),
        })
    res = run_bass_kernel_spmd(_NC, in_maps, list(range(8)))
    y = np.empty((B, S, D), dtype=np.float32)
    for b in range(B):
        acc = res.results[4 * b]["y"].astype(np.float32)
        for g in range(1, G):
            acc = acc + res.results[4 * b + g]["y"]
        y[b] = acc
    return y



# revision 2
# speedup vs baseline: 1.7120x; 1.7120x over previous
"""Causal self-attention (B=2, S=2048, D=1024, H=16) on 8 NeuronCores.

Sharding (per spec hint): data-parallel over batch (2 groups of 4 cores),
tensor-parallel over heads within a group (4 heads / core). Each core
computes Q/K/V projections for its 4 heads, causal flash-style attention,
and a partial output projection through its slice of Wo. The 4 partial
[2048, 1024] outputs per batch are summed on the host (unsharding step).

Per-core kernel layout notes:
  - All activations kept feature-major ("transposed"): xT [1024, 2048],
    QT/KT [256, 2048]. Scores are computed transposed, ST[k, q], so the
    P@V contraction (over k) needs no transposes anywhere.
  - Softmax skips the max-subtraction (scores ~ N(0,1); exp can't
    overflow fp32) so exp is a single ACT pass straight out of PSUM.
  - The softmax denominator rides along the P@V matmul as a fused
    ones-column in the V operand (M=65), then gets broadcast across
    partitions with a K=1 matmul for the normalize multiply.
  - Matmuls run in float32r (TF32-like, full PE rate; measured rms rel
    err 1.5e-4 at K=1024 on HW) with fp32 PSUM accumulation.
"""

import numpy as np

import concourse.bass as bass
import concourse.mybir as mybir
import concourse.tile as tile
from concourse.bass_utils import run_bass_kernel_spmd

F32 = mybir.dt.float32
F32R = mybir.dt.float32r
AF = mybir.ActivationFunctionType

B, S, D, H = 2, 2048, 1024, 16
DH = D // H              # 64
HL = 4                   # heads per core
CL = HL * DH             # 256 channels per core
G = 4                    # cores per batch group
SCALE = DH ** -0.5       # 0.125
NQC = S // 512           # 4 q-chunks of 512
NKT = S // 128           # 16 key tiles of 128


def _split_excess_waits(nc, max_waits=1):
    """walrus in this toolchain rejects instructions carrying more than
    `max_waits` sem waits; split the excess onto preceding same-engine
    NoOps (sound: waits are monotone >= conditions hoisted earlier on
    the same engine)."""
    n_split = 0
    for f in nc.m.functions:
        for bb in f.blocks:
            out = []
            for inst in bb.instructions:
                si = inst.sync_info
                waits = list(si.on_wait) if si is not None and si.on_wait else []
                if len(waits) > max_waits:
                    head, keep = waits[:-max_waits], waits[-max_waits:]
                    for ci, start in enumerate(range(0, len(head), max_waits)):
                        nop = mybir.InstNoOp(
                            name=f"{inst.name}_wsplit{ci}",
                            sync_info=mybir.SyncInfo(
                                on_wait=head[start:start + max_waits],
                                on_update=[],
                            ),
                            engine=inst.engine,
                            bass_nofuse=True,
                        )
                        out.append(nop)
                        n_split += 1
                    si.on_wait = keep
                out.append(inst)
            if n_split:
                bb.instructions.clear()
                for i in out:
                    bb.instructions.append(i)
    return n_split


def _build_nc(split_waits=True):
    nc = bass.Bass()
    xt_d = nc.dram_tensor("xt", [D, S], F32, kind="ExternalInput")
    wq_d = nc.dram_tensor("wq", [D, CL], F32, kind="ExternalInput")
    wk_d = nc.dram_tensor("wk", [D, CL], F32, kind="ExternalInput")
    wv_d = nc.dram_tensor("wv", [D, CL], F32, kind="ExternalInput")
    wo_d = nc.dram_tensor("wo", [128, 2, D], F32, kind="ExternalInput")
    mask_d = nc.dram_tensor("mask", [128, 2, 128], F32, kind="ExternalInput")
    y_d = nc.dram_tensor("y", [S, D], F32, kind="ExternalOutput")

    with tile.TileContext(nc) as tc:
        with tc.tile_pool(name="persist", bufs=1) as pp:
            # ---- persistent SBUF tensors -------------------------------
            wo_sb = pp.tile([128, 2, D], F32R)   # pair-major k-tiles
            mask_sb = pp.tile([128, 2, 128], F32)     # tri m[k,q]=k<=q, x2 heads
            ones_f = pp.tile([128, 128], F32)
            zeros_f = pp.tile([128, 768], F32)
            ones_sb = pp.tile([128, 128], F32R)
            qt_sb = [pp.tile([128, S], F32R, name=f"qt{p}", tag=f"qt{p}")
                     for p in range(2)]
            kt_sb = [pp.tile([128, S], F32R, name=f"kt{p}", tag=f"kt{p}")
                     for p in range(2)]
            # V' per key-tile: 4x[64 v-cols + 1 ones-col]
            vp_sb = pp.tile([128, NKT, 4 * 65], F32R)

            nc.vector.memset(ones_f[:], 1.0)
            nc.vector.memset(zeros_f[:], 0.0)
            nc.vector.tensor_copy(ones_sb[:], ones_f[:])
            for hl in range(4):
                nc.vector.tensor_copy(
                    vp_sb[:, :, hl * 65 + 64:hl * 65 + 65], ones_f[:, 0:NKT])

            # ---- phase 1: projections ---------------------------------
            # QT/KT [256, S] = W.T-slice.T @ xT ; V [S, 256] = xT.T @ wv
            with (
                tc.tile_pool(name="ph1", bufs=1) as ph1,
                tc.tile_pool(name="pj", bufs=2, space="PSUM") as pj,
            ):
                xt_sb = ph1.tile([128, 8, S], F32R)        # x.T, k-tiled
                wq_sb = ph1.tile([128, 8, CL], F32R)
                wk_sb = ph1.tile([128, 8, CL], F32R)
                wv_sb = ph1.tile([128, 8, CL], F32R)
                xt_r = xt_d.rearrange("(a p) s -> p a s", p=128).bitcast(F32R)
                nc.sync.dma_start(
                    wq_sb[:],
                    wq_d.rearrange("(a p) m -> p a m", p=128).bitcast(F32R))
                nc.sync.dma_start(
                    wk_sb[:],
                    wk_d.rearrange("(a p) m -> p a m", p=128).bitcast(F32R))
                nc.sync.dma_start(
                    wv_sb[:],
                    wv_d.rearrange("(a p) m -> p a m", p=128).bitcast(F32R))
                for k in range(8):  # per-k chunks so matmuls start early
                    nc.sync.dma_start(xt_sb[:, k, :], xt_r[:, k, :])
                # wo/mask are not needed until mid-attention; load after xT
                nc.sync.dma_start(wo_sb[:], wo_d[:, :, :].bitcast(F32R))
                nc.sync.dma_start(mask_sb[:], mask_d[:, :, :])
                # q-chunk-major so attention on early chunks starts sooner
                for c in range(NQC):
                    cslc = slice(c * 512, (c + 1) * 512)
                    for p in range(2):
                        pslc = slice(p * 128, (p + 1) * 128)
                        psq = pj.tile([128, 512], F32, tag="pq", bufs=3)
                        psk = pj.tile([128, 512], F32, tag="pk", bufs=3)
                        for k in range(8):
                            nc.tensor.matmul(
                                psq[:], wq_sb[:, k, pslc], xt_sb[:, k, cslc],
                                start=(k == 0), stop=(k == 7))
                        for k in range(8):
                            nc.tensor.matmul(
                                psk[:], wk_sb[:, k, pslc], xt_sb[:, k, cslc],
                                start=(k == 0), stop=(k == 7))
                        nc.scalar.copy(qt_sb[p][:, cslc], psq[:])
                        nc.scalar.copy(kt_sb[p][:, cslc], psk[:])
                    for st in range(4 * c, 4 * (c + 1)):
                        psv = pj.tile([128, CL], F32, tag="pv")
                        for k in range(8):
                            nc.tensor.matmul(
                                psv[:], xt_sb[:, k, st * 128:(st + 1) * 128],
                                wv_sb[:, k, :], start=(k == 0), stop=(k == 7))
                        nc.scalar.copy(
                            vp_sb[:, st, :]
                            .rearrange("p (h e) -> p h e", e=65)[:, :, 0:64],
                            psv[:].rearrange("p (h d) -> p h d", d=64))

            # ---- phase 2/3: attention + out-projection ----------------
            with (
                tc.tile_pool(name="stp", bufs=2, space="PSUM") as stp,
                tc.tile_pool(name="otp", bufs=2, space="PSUM") as otp,
                tc.tile_pool(name="pt", bufs=6) as ptp,
                tc.tile_pool(name="nrm", bufs=2) as nrm,
                tc.tile_pool(name="osb", bufs=4) as osb,
            ):
                for qc in range(NQC):
                    qlo = qc * 512
                    qslc = slice(qlo, qlo + 512)
                    os_tiles = []           # one [64, 512] tile per local head
                    for p in range(2):
                        OTP = otp.tile([65, 2, 512], F32, tag="ot")
                        ktmax = 4 * (qc + 1)
                        for kt in range(ktmax):
                            first, last = kt == 0, kt == ktmax - 1
                            ST = stp.tile([128, 2, 512], F32, tag="st")
                            for hi in range(2):
                                hslc = slice(hi * 64, (hi + 1) * 64)
                                nc.tensor.matmul(
                                    ST[:, hi, :],
                                    kt_sb[p][hslc, kt * 128:(kt + 1) * 128],
                                    qt_sb[p][hslc, qslc],
                                    start=True, stop=True)
                            PT = ptp.tile([128, 2, 512], F32R, tag="pt")
                            dq = max(0, kt * 128 - qlo)
                            nc.scalar.activation(PT[:, :, dq:], ST[:, :, dq:],
                                                 AF.Exp, scale=SCALE)
                            if kt * 128 >= qlo:  # diagonal: mask keys > query
                                if dq > 0:
                                    nc.vector.tensor_copy(
                                        PT[:, :, 0:dq], zeros_f[:, 0:2 * dq])
                                nc.vector.tensor_mul(
                                    PT[:, :, dq:dq + 128],
                                    PT[:, :, dq:dq + 128], mask_sb[:])
                            # P@V (transposed): OT[c, q] += [V|1].T @ PT
                            # row 64 of each head region = softmax denominator
                            for hi in range(2):
                                bc = (2 * p + hi) * 65
                                nc.tensor.matmul(
                                    OTP[0:65, hi, :], vp_sb[:, kt, bc:bc + 65],
                                    PT[:, hi, :], start=first, stop=last)
                        # normalize: rows 0:64 of each head / its denom row 64
                        Ri = nrm.tile([128, 2, 512], F32R, tag="ri")
                        with nc.allow_low_precision(reason="softmax recip"):
                            nc.vector.reciprocal(Ri[64:65, :, :],
                                                 OTP[64:65, :, :])
                        OC = osb.tile([64, 2, 512], F32, tag="oc")
                        nc.vector.tensor_copy(OC[:, :, :], OTP[0:64, :, :])
                        # pack the head pair into one [128, 512] k-tile for
                        # the out-projection: even head normalizes in place,
                        # odd head normalizes to a scratch tile and is moved
                        # to partitions 64:128 by an SBUF-to-SBUF DMA.
                        OS = osb.tile([128, 512], F32R, name="OS", tag=f"os{p}")
                        OSm = osb.tile([64, 512], F32R, name="OSm", tag="osm")
                        Rb = stp.tile([128, 2, 512], F32, name="Rb", tag="st")
                        for hi in range(2):
                            nc.tensor.matmul(
                                Rb[:, hi, :], ones_sb[64:65, :],
                                Ri[64:65, hi, :], start=True, stop=True)
                        nc.vector.tensor_mul(OS[0:64, :], OC[:, 0, :],
                                             Rb[0:64, 0, :])
                        nc.vector.tensor_mul(OSm[:, :], OC[:, 1, :],
                                             Rb[0:64, 1, :])
                        nc.sync.dma_start(OS[64:128, :], OSm[:, :])
                        os_tiles.append(OS)
                    # out-projection for this q-chunk: accumulate over 2 pairs
                    for st4 in range(4):
                        sslc = slice(st4 * 128, (st4 + 1) * 128)
                        for nch in range(2):
                            yp = otp.tile([128, 512], F32, name="yp", tag="ot")
                            for kp in range(2):
                                nc.tensor.matmul(
                                    yp[:], os_tiles[kp][:, sslc],
                                    wo_sb[:, kp, nch * 512:(nch + 1) * 512],
                                    start=(kp == 0), stop=(kp == 1))
                            ysb = osb.tile([128, 512], F32, name="ysb", tag="ys")
                            nc.vector.tensor_copy(ysb[:], yp[:])
                            nc.sync.dma_start(
                                y_d[qlo + st4 * 128:qlo + (st4 + 1) * 128,
                                    nch * 512:(nch + 1) * 512], ysb[:])

    if split_waits:
        _split_excess_waits(nc, max_waits=1)
    return nc


_NC = None


def _in_maps(x, Wq, Wk, Wv, Wo):
    x = np.asarray(x, dtype=np.float32)
    Wq, Wk, Wv, Wo = (np.asarray(w, dtype=np.float32) for w in (Wq, Wk, Wv, Wo))
    tri = np.triu(np.ones((128, 128), dtype=np.float32))  # m[k,q] = k<=q
    in_maps = []
    for core in range(8):
        b, g = divmod(core, G)
        csl = slice(g * CL, (g + 1) * CL)
        in_maps.append({
            "xt": np.ascontiguousarray(x[b].T),
            "wq": np.ascontiguousarray(Wq[csl, :].T),
            "wk": np.ascontiguousarray(Wk[csl, :].T),
            "wv": np.ascontiguousarray(Wv[csl, :].T),
            "wo": np.ascontiguousarray(
                Wo[:, csl].T.reshape(2, 128, D).transpose(1, 0, 2)),
            "mask": np.ascontiguousarray(np.stack([tri, tri], axis=1)),
        })
    return in_maps


def _sim_inputs(inputs):
    return _in_maps(**inputs)[0]


def kernel(x, Wq, Wk, Wv, Wo):
    global _NC
    if _NC is None:
        _NC = _build_nc()
    in_maps = _in_maps(x, Wq, Wk, Wv, Wo)
    res = run_bass_kernel_spmd(_NC, in_maps, list(range(8)))
    y = np.empty((B, S, D), dtype=np.float32)
    for b in range(B):
        acc = res.results[4 * b]["y"].astype(np.float32)
        for g in range(1, G):
            acc = acc + res.results[4 * b + g]["y"]
        y[b] = acc
    return y

